# revision 39
# baseline (speedup 1.0000x reference)
"""Trainium2 Bass kernel for nn_BCE_Loss (focal-style BCE-with-logits, mean).

Reference math per anchor row x[0:3] (logits) and integer target c:
    col = 0 if c==1 else 1 if c==3 else 2
    t   = one_hot(col, 3)
    w   = (1-pt)^2,  pt = x*t + (1-x)*(1-t)        [from detached logits]
    bce = max(x,0) - x*t + log1p(exp(-|x|))
    out = mean(w * bce)

Per element the reference equals u^2 * softplus(v) with v = x*(1-2t),
u = v + t.  Two exact identities (for t in {0,1}) remove every
target-dependent elementwise tensor:
    u^2 = x^2 + u - x          (u(u-1) == x(x-1) in both branches)
    softplus(v) = S - x*t,     S := softplus(x)   (sp(x)-sp(-x) == x)
which collapse the loss to
    sum loss = sum x^2*S  +  sum_am t[a,m] * q(x[a,m])
    q(x) = (1-2x)*S - x*(x-1)^2 = -2*(x*S) + (S - x*(x-1)^2)
so the activation path runs on RAW logits (no dependency on the target
path) and the one-hot enters only through per-class planes t_m [P,T]
built contiguously (never a strided interleaved write).

Layout/engine split per core (NT tiles of [P=128, F=3T]):
    one-hot planes for the WHOLE shard are built up front from one bf16
    target load: t0=(c==1), t1=(c==3), t2=(1-t0)-t1  (3 DVE ops total)
    per tile:
      ACT:  E = Exp(x), S = Ln(E+1)          [+ (x-1)^2 on ACT_SQ tiles]
      DVE:  XS = x*S,  B = x(x-1)^2,  M = S - B
      PE:   three accumulating diag-trick banks (one PSUM bank each --
            start=True clears a whole physical bank):
              bank0 += x_c^T @ XS_c      (diag: sum x^2 S)
              bank1 += t_mc^T @ XS_mc    (diag: sum t x S, strided moving)
              bank2 += t_mc^T @ M_mc     (diag: sum t (S - B))
            total = diag0 - 2*diag1 + diag2 via a host coefficient mask.

HBM traffic per core: pred 12.58 MB (f32, cast to bf16 in the SWDGE DMA
datapath) + targ 2.1 MB (host-narrowed int64 -> bf16; values 0..4 are
exact in bf16) ~= 14.7 MB.

Sharding: pure data-parallel across 8 NeuronCores -- each core takes a
contiguous block of anchors; per-core output is a single partial sum; the
host sums the 8 partials and divides by the element count.
"""

import numpy as np

import concourse.bacc as bacc
import concourse.bass as bass
import concourse.mybir as mybir
from concourse import bass_utils
from concourse.alu_op_type import AluOpType
from concourse.tile import TileContext

N_CORES = 8
N_ANCHORS = 8388608
N_CLASSES = 3
N_SHARD = N_ANCHORS // N_CORES  # 1048576
P = 128  # SBUF partitions
T = 1024  # anchor rows per partition per tile
NT = N_SHARD // (P * T)  # 8 tiles per core
F = N_CLASSES * T  # free dim of an x tile (3072)
TT = NT * T  # anchors per partition for the whole shard (8192)
MM = 128  # diag-trick matmul chunk width
NB = 3  # psum banks (coefficient groups)
# tiles whose (x-1)^2 runs on ACT (A/B-tuned): tile 0's quarters' small
# squares fill the early ACT starvation gaps, mid-tiles {1,3,5} balance
# DVE vs ACT, and tile 7 keeps its halves from leaving a DVE chain
# dangling in the drain
ACT_SQ_TILES = {0, 1, 3, 5, 7}


class _Bacc(bacc.Bacc):
    """Bacc with the ACT table pinned to natural_log_exp_and_others.

    The default chooser puts Exp in exp_and_others and Ln in natural_log,
    reloading tables every tile (~1.3us each). Exp, Ln and Square all live
    in natural_log_exp_and_others; emptying every other set (positions
    kept -- act_func_set_id is the index into act_info.json) forces one
    load."""

    _ACT_SET = "natural_log_exp_and_others"

    def insert_act_table_loads(self):
        import bass_rust as _bass_rust

        from concourse.hw_specs import get_activation_tables

        has_activation = any(
            isinstance(i, mybir.InstActivation)
            for b in self.main_func.blocks
            for i in b.instructions
        )
        if not has_activation:
            return
        tables = [
            (name, (fns if name == self._ACT_SET else set()))
            for name, fns in get_activation_tables(self.m.arch).items()
        ]
        _bass_rust.insert_act_table_loads(self, tables)


def _build_nc() -> bass.Bass:
    nc = _Bacc("TRN2", target_bir_lowering=False, num_swdge_queues=4)
    # the Square(x, bias=-1) activation needs a -1.0 const AP; only 0/1 ship
    _m1 = nc.alloc_sbuf_tensor("const-float32--1.0", [128, 1], mybir.dt.float32)
    nc.gpsimd.memset(_m1.ap(), -1.0)
    nc.const_aps.aps[(mybir.dt.float32, -1.0)] = _m1.ap()
    pred = nc.dram_tensor(
        "pred", [N_SHARD, N_CLASSES], mybir.dt.float32, kind="ExternalInput"
    )
    targ = nc.dram_tensor("targb", [N_SHARD], mybir.dt.bfloat16, kind="ExternalInput")
    cmask = nc.dram_tensor(
        "cmask", [P, NB * MM], mybir.dt.bfloat16, kind="ExternalInput"
    )
    out = nc.dram_tensor("out", [1], mybir.dt.float32, kind="ExternalOutput")

    xv = pred.rearrange("(n p t) m -> n p (t m)", p=P, t=T)
    tv = targ.rearrange("(p t) -> p t", p=P)  # [P, TT], whole shard

    n_mm = F // MM  # interleaved chunks per tile (24)
    n_tm = T // MM  # per-plane chunks per tile (8)

    with TileContext(nc) as tc:
        with (
            tc.tile_pool(name="io", bufs=4) as io,
            tc.tile_pool(name="stat", bufs=1) as stat,
            tc.tile_pool(name="epool", bufs=2) as epool,
            tc.tile_pool(name="spool", bufs=2) as spool,
            tc.tile_pool(name="xsp", bufs=2) as xsp,
            tc.tile_pool(name="mpool", bufs=2) as mpool,
            tc.tile_pool(name="bpool", bufs=2) as bpool,
            tc.tile_pool(name="scratch", bufs=1) as scr,
            tc.tile_pool(name="singles", bufs=1) as singles,
            tc.tile_pool(name="psum", bufs=1, space="PSUM") as psum,
        ):
            ones_f = singles.tile([P, 1], mybir.dt.float32)
            nc.vector.memset(ones_f, 1.0)
            pb0 = psum.tile([P, 512], mybir.dt.float32)
            pb1 = psum.tile([P, 512], mybir.dt.float32)
            pb2 = psum.tile([P, 512], mybir.dt.float32)

            # target load first (split in two halves so the first half's
            # one-hot planes are ready before tile 0's T-bank matmuls), then
            # every x cast-DMA trigger
            # the very first x quarter rides the HWDGE sync ring (which gets
            # its first packet out ~3us earlier than SWDGE) as raw f32 and is
            # cast to bf16 by the otherwise-idle DVE
            xq0_f32 = stat.tile([P, N_CLASSES * (T // 4)], mybir.dt.float32)
            nc.sync.dma_start(out=xq0_f32, in_=pred.rearrange(
                "(p t) m -> p (t m)", p=P)[:, : N_CLASSES * (T // 4)])
            cm = singles.tile([P, NB * MM], mybir.dt.bfloat16)
            nc.sync.dma_start(out=cm, in_=cmask[:, :])
            tg = stat.tile([P, TT], mybir.dt.bfloat16)
            HH = TT // 2
            nc.sync.dma_start(out=tg[:, :HH], in_=tv[:, :HH])
            # piece list: tile 0 split into quarters so the first Exp starts
            # as soon as ~0.4 MB (not 1.57 MB) has landed; rest full tiles.
            # each piece: (tile_idx, anchor_offset_within_shard, width_T)
            pieces = [(0, q * (T // 4), T // 4) for q in range(4)]
            pieces += [(i, i * T, T) for i in range(1, NT - 1)]
            pieces += [(NT - 1, (NT - 1) * T + q * (T // 2), T // 2)
                       for q in range(2)]
            xvf = pred.rearrange("(p t) m -> p (t m)", p=P)  # whole shard view
            xbs = []
            for pidx, (ti, off, tw) in enumerate(pieces):
                xb = io.tile([P, N_CLASSES * tw], mybir.dt.bfloat16)
                if pidx == 0:
                    nc.vector.tensor_copy(out=xb, in_=xq0_f32)
                else:
                    nc.gpsimd.dma_start(
                        out=xb, in_=xvf[:, N_CLASSES * off : N_CLASSES * (off + tw)])
                xbs.append(xb)

            # one-hot planes, built per half in DVE's idle window; the odd
            # plane via mod (t2 = 1 - (c mod 2): even class ids are class 2)
            tp = stat.tile([P, N_CLASSES * TT], mybir.dt.bfloat16)
            tp3 = tp.rearrange("p (m t) -> p m t", m=N_CLASSES)
            n01 = stat.tile([P, HH], mybir.dt.bfloat16)

            def build_planes(h):
                hs = slice(h * HH, (h + 1) * HH)
                nc.vector.tensor_scalar(
                    out=tp3[:, 0, hs], in0=tg[:, hs], scalar1=1.0, scalar2=0.0,
                    op0=AluOpType.is_equal, op1=AluOpType.add)
                nc.vector.tensor_scalar(
                    out=tp3[:, 1, hs], in0=tg[:, hs], scalar1=3.0, scalar2=0.0,
                    op0=AluOpType.is_equal, op1=AluOpType.add)
                nc.vector.tensor_tensor(
                    out=n01, in0=tp3[:, 0, hs], in1=tp3[:, 1, hs],
                    op=AluOpType.add)
                nc.vector.tensor_scalar(
                    out=tp3[:, 2, hs], in0=n01, scalar1=-1.0, scalar2=1.0,
                    op0=AluOpType.mult, op1=AluOpType.add)

            build_planes(0)

            for pi, (ti, off, tw) in enumerate(pieces):
                xb = xbs[pi]
                fw = N_CLASSES * tw

                # E = exp(x) ; S = ln(E + 1) = softplus(x)  (raw logits)
                E = epool.tile([P, fw], mybir.dt.bfloat16)
                nc.scalar.activation(
                    out=E, in_=xb, func=mybir.ActivationFunctionType.Exp)
                S = spool.tile([P, fw], mybir.dt.bfloat16)
                nc.scalar.activation(
                    out=S, in_=E, func=mybir.ActivationFunctionType.Ln, bias=1.0)
                if pi == 0:
                    # second target half rides the ACT HWDGE ring so it does
                    # not steal early DMA bandwidth from tile 0's x
                    nc.scalar.dma_start(out=tg[:, HH:], in_=tv[:, HH:])
                if ti == 2 and off == 2 * T:
                    build_planes(1)

                # B = x(x-1)^2: (x-1)^2 from ACT Square on ACT_SQ tiles,
                # else a ts/tt chain on DVE (scalar_tensor_tensor is 1x-only)
                B = bpool.tile([P, fw], mybir.dt.bfloat16)
                if ti in ACT_SQ_TILES:
                    sq = scr.tile([P, fw], mybir.dt.bfloat16)
                    nc.scalar.activation(
                        out=sq, in_=xb,
                        func=mybir.ActivationFunctionType.Square, bias=-1.0)
                    nc.vector.tensor_tensor(
                        out=B, in0=xb, in1=sq, op=AluOpType.mult)
                else:
                    xm1 = scr.tile([P, fw], mybir.dt.bfloat16)
                    nc.vector.tensor_scalar(
                        out=xm1, in0=xb, scalar1=1.0, scalar2=0.0,
                        op0=AluOpType.subtract, op1=AluOpType.add)
                    b1 = scr.tile([P, fw], mybir.dt.bfloat16)
                    nc.vector.tensor_tensor(
                        out=b1, in0=xm1, in1=xb, op=AluOpType.mult)
                    nc.vector.tensor_tensor(
                        out=B, in0=xm1, in1=b1, op=AluOpType.mult)

                # XS = x*S ; M = S - B
                XS = xsp.tile([P, fw], mybir.dt.bfloat16)
                nc.vector.tensor_tensor(out=XS, in0=xb, in1=S, op=AluOpType.mult)
                M = mpool.tile([P, fw], mybir.dt.bfloat16)
                nc.vector.tensor_tensor(out=M, in0=S, in1=B, op=AluOpType.subtract)

                # strided per-class views [P, tw, m] and this piece's planes
                XS3 = XS.rearrange("p (t m) -> p t m", m=N_CLASSES)
                M3 = M.rearrange("p (t m) -> p t m", m=N_CLASSES)

                first, last = (pi == 0), (pi == len(pieces) - 1)
                # bank0: x^T @ XS (interleaved, contiguous chunks)
                for c in range(fw // MM):
                    s = slice(c * MM, (c + 1) * MM)
                    nc.tensor.matmul(
                        pb0[:, :MM], xb[:, s], XS[:, s],
                        start=(first and c == 0),
                        stop=(last and c == fw // MM - 1),
                        skip_group_check=True)
                # bank1: t^T @ XS ; bank2: t^T @ M
                for m in range(N_CLASSES):
                    for c in range(tw // MM):
                        s = slice(c * MM, (c + 1) * MM)
                        sg = slice(off + c * MM, off + (c + 1) * MM)
                        nc.tensor.matmul(
                            pb1[:, :MM], tp3[:, m, sg], XS3[:, s, m],
                            start=(first and m == 0 and c == 0),
                            stop=(last and m == N_CLASSES - 1 and c == tw // MM - 1),
                            skip_group_check=True)
                        nc.tensor.matmul(
                            pb2[:, :MM], tp3[:, m, sg], M3[:, s, m],
                            start=(first and m == 0 and c == 0),
                            stop=(last and m == N_CLASSES - 1 and c == tw // MM - 1),
                            skip_group_check=True)

            # epilogue: total = sum_k coef_k * diag(bank_k) via host mask
            dA = singles.tile([P, NB * MM], mybir.dt.float32)
            dA4 = dA.rearrange("p (k c) -> p k c", k=NB)
            cm4 = cm.rearrange("p (k c) -> p k c", k=NB)
            pbanks = [pb0, pb1, pb2]
            for k in range(NB):
                nc.vector.tensor_tensor(
                    out=dA4[:, k, :], in0=pbanks[k][:, :MM], in1=cm4[:, k, :],
                    op=AluOpType.mult)
            rA = singles.tile([P, 1], mybir.dt.float32)
            nc.vector.tensor_reduce(
                out=rA, in_=dA, axis=mybir.AxisListType.X, op=AluOpType.add)

            psT = psum.tile([1, 1], mybir.dt.float32)
            nc.tensor.matmul(psT[:, :], ones_f[:, :], rA[:, :], start=True, stop=True)
            res = singles.tile([1, 1], mybir.dt.float32)
            nc.vector.tensor_copy(out=res, in_=psT)
            nc.sync.dma_start(out=out[:], in_=res[0, :])

    nc.compile()
    return nc


_cache: dict[str, bass.Bass] = {}
last_results = None  # BassKernelResults of the most recent run (for test.py)


def _get_nc() -> bass.Bass:
    if "nc" not in _cache:
        _cache["nc"] = _build_nc()
    return _cache["nc"]


def _cmask_bf16() -> np.ndarray:
    import ml_dtypes

    coef = np.array([1.0, -2.0, 1.0], dtype=np.float32)
    m = np.zeros((P, NB * MM), dtype=np.float32)
    for k in range(NB):
        m[:, k * MM : (k + 1) * MM] = coef[k] * np.eye(P, MM, dtype=np.float32)
    return m.astype(ml_dtypes.bfloat16)


def kernel(pred: np.ndarray, targ: np.ndarray, *, trace: bool = False) -> np.ndarray:
    global last_results
    import ml_dtypes

    pred = np.ascontiguousarray(np.asarray(pred, dtype=np.float32))
    targ = np.asarray(targ)
    assert pred.shape == (N_ANCHORS, N_CLASSES), pred.shape
    assert targ.shape == (N_ANCHORS,), targ.shape

    # targ holds class ids 0..4 -- bf16 is a lossless narrowing that cuts
    # the on-device HBM read from 8.4 MB to 2.1 MB per core and keeps the
    # on-chip compares in the DVE's fast all-bf16 mode
    targb = np.ascontiguousarray(targ.astype(ml_dtypes.bfloat16))

    nc = _get_nc()
    cmask = _cmask_bf16()

    in_maps = []
    for c in range(N_CORES):
        in_maps.append({
            "pred": pred[c * N_SHARD : (c + 1) * N_SHARD],
            "targb": targb[c * N_SHARD : (c + 1) * N_SHARD],
            "cmask": cmask,
        })

    res = bass_utils.run_bass_kernel_spmd(
        nc, in_maps, core_ids=list(range(N_CORES)), trace=trace
    )
    last_results = res

    total = np.float64(0.0)
    for r in res.results:
        total += np.float64(r["out"][0])
    mean = total / (N_ANCHORS * N_CLASSES)
    return np.float32(mean)


# revision 40
# speedup vs baseline: 1.1907x; 1.1907x over previous
"""Trainium2 Bass kernel for nn_BCE_Loss (focal-style BCE-with-logits, mean).

Reference math per anchor row x[0:3] (logits) and integer target c:
    col = 0 if c==1 else 1 if c==3 else 2
    t   = one_hot(col, 3)
    w   = (1-pt)^2,  pt = x*t + (1-x)*(1-t)        [from detached logits]
    bce = max(x,0) - x*t + log1p(exp(-|x|))
    out = mean(w * bce)

Per element the reference equals u^2 * softplus(v) with v = x*(1-2t),
u = v + t.  Two exact identities (for t in {0,1}) remove every
target-dependent elementwise tensor:
    u^2 = x^2 + u - x          (u(u-1) == x(x-1) in both branches)
    softplus(v) = S - x*t,     S := softplus(x)   (sp(x)-sp(-x) == x)
which collapse the loss to
    sum loss = sum x^2*S  +  sum_am t[a,m] * q(x[a,m])
    q(x) = (1-2x)*S - x*(x-1)^2 = -2*(x*S) + (S - x*(x-1)^2)
so the activation path runs on RAW logits (no dependency on the target
path) and the one-hot enters only through per-class planes t_m [P,T]
built contiguously (never a strided interleaved write).

Layout/engine split per core (NT tiles of [P=128, F=3T]):
    one-hot planes for the WHOLE shard are built up front from one bf16
    target load: t0=(c==1), t1=(c==3), t2=(1-t0)-t1  (3 DVE ops total)
    per tile:
      ACT:  E = Exp(x), S = Ln(E+1)          [+ (x-1)^2 on ACT_SQ tiles]
      DVE:  XS = x*S,  B = x(x-1)^2,  M = S - B
      PE:   three accumulating diag-trick banks (one PSUM bank each --
            start=True clears a whole physical bank):
              bank0 += x_c^T @ XS_c      (diag: sum x^2 S)
              bank1 += t_mc^T @ XS_mc    (diag: sum t x S, strided moving)
              bank2 += t_mc^T @ M_mc     (diag: sum t (S - B))
            total = diag0 - 2*diag1 + diag2 via a host coefficient mask.

HBM traffic per core: pred 12.58 MB (f32, cast to bf16 in the SWDGE DMA
datapath) + targ 2.1 MB (host-narrowed int64 -> bf16; values 0..4 are
exact in bf16) ~= 14.7 MB.

Sharding: pure data-parallel across 8 NeuronCores -- each core takes a
contiguous block of anchors; per-core output is a single partial sum; the
host sums the 8 partials and divides by the element count.
"""

import numpy as np

import concourse.bacc as bacc
import concourse.bass as bass
import concourse.mybir as mybir
from concourse import bass_utils
from concourse.alu_op_type import AluOpType
from concourse.tile import TileContext

N_CORES = 8
N_ANCHORS = 8388608
N_CLASSES = 3
N_SHARD = N_ANCHORS // N_CORES  # 1048576
P = 128  # SBUF partitions
T = 1024  # anchor rows per partition per tile
NT = N_SHARD // (P * T)  # 8 tiles per core
F = N_CLASSES * T  # free dim of an x tile (3072)
TT = NT * T  # anchors per partition for the whole shard (8192)
MM = 128  # diag-trick matmul chunk width
NB = 3  # psum banks (coefficient groups)
# tiles whose (x-1)^2 runs on ACT (A/B-tuned): tile 0's quarters' small
# squares fill the early ACT starvation gaps, mid-tiles {1,3,5} balance
# DVE vs ACT, and tile 7 keeps its halves from leaving a DVE chain
# dangling in the drain
ACT_SQ_TILES = {0, 1, 3, 5, 7}


class _Bacc(bacc.Bacc):
    """Bacc with the ACT table pinned to natural_log_exp_and_others.

    The default chooser puts Exp in exp_and_others and Ln in natural_log,
    reloading tables every tile (~1.3us each). Exp, Ln and Square all live
    in natural_log_exp_and_others; emptying every other set (positions
    kept -- act_func_set_id is the index into act_info.json) forces one
    load."""

    _ACT_SET = "natural_log_exp_and_others"

    def insert_act_table_loads(self):
        import bass_rust as _bass_rust

        from concourse.hw_specs import get_activation_tables

        has_activation = any(
            isinstance(i, mybir.InstActivation)
            for b in self.main_func.blocks
            for i in b.instructions
        )
        if not has_activation:
            return
        tables = [
            (name, (fns if name == self._ACT_SET else set()))
            for name, fns in get_activation_tables(self.m.arch).items()
        ]
        _bass_rust.insert_act_table_loads(self, tables)


def _build_nc() -> bass.Bass:
    nc = _Bacc("TRN2", target_bir_lowering=False, num_swdge_queues=4)
    # the Square(x, bias=-1) activation needs a -1.0 const AP; only 0/1 ship
    _m1 = nc.alloc_sbuf_tensor("const-float32--1.0", [128, 1], mybir.dt.float32)
    nc.gpsimd.memset(_m1.ap(), -1.0)
    nc.const_aps.aps[(mybir.dt.float32, -1.0)] = _m1.ap()
    pred = nc.dram_tensor(
        "pred", [N_SHARD, N_CLASSES], mybir.dt.float32, kind="ExternalInput"
    )
    targ = nc.dram_tensor("targb", [N_SHARD], mybir.dt.bfloat16, kind="ExternalInput")
    cmask = nc.dram_tensor(
        "cmask", [P, NB * MM], mybir.dt.bfloat16, kind="ExternalInput"
    )
    out = nc.dram_tensor("out", [1], mybir.dt.float32, kind="ExternalOutput")

    xv = pred.rearrange("(n p t) m -> n p (t m)", p=P, t=T)
    tv = targ.rearrange("(p t) -> p t", p=P)  # [P, TT], whole shard

    n_mm = F // MM  # interleaved chunks per tile (24)
    n_tm = T // MM  # per-plane chunks per tile (8)

    with TileContext(nc) as tc:
        with (
            tc.tile_pool(name="io", bufs=5) as io,
            tc.tile_pool(name="stat", bufs=1) as stat,
            tc.tile_pool(name="epool", bufs=2) as epool,
            tc.tile_pool(name="spool", bufs=2) as spool,
            tc.tile_pool(name="xsp", bufs=2) as xsp,
            tc.tile_pool(name="mpool", bufs=2) as mpool,
            tc.tile_pool(name="bpool", bufs=2) as bpool,
            tc.tile_pool(name="scratch", bufs=1) as scr,
            tc.tile_pool(name="singles", bufs=1) as singles,
            tc.tile_pool(name="psum", bufs=1, space="PSUM") as psum,
        ):
            ones_f = singles.tile([P, 1], mybir.dt.float32)
            nc.vector.memset(ones_f, 1.0)
            pb0 = psum.tile([P, 512], mybir.dt.float32)
            pb1 = psum.tile([P, 512], mybir.dt.float32)
            pb2 = psum.tile([P, 512], mybir.dt.float32)

            # target load first (split in two halves so the first half's
            # one-hot planes are ready before tile 0's T-bank matmuls), then
            # every x cast-DMA trigger
            # the very first x quarter rides the HWDGE sync ring (which gets
            # its first packet out ~3us earlier than SWDGE) as raw f32 and is
            # cast to bf16 by the otherwise-idle DVE
            xq0_f32 = stat.tile([P, N_CLASSES * (T // 4)], mybir.dt.float32)
            nc.sync.dma_start(out=xq0_f32, in_=pred.rearrange(
                "(p t) m -> p (t m)", p=P)[:, : N_CLASSES * (T // 4)])
            cm = singles.tile([P, NB * MM], mybir.dt.bfloat16)
            nc.sync.dma_start(out=cm, in_=cmask[:, :])
            tg = stat.tile([P, TT], mybir.dt.bfloat16)
            HH = TT // 2
            nc.sync.dma_start(out=tg[:, :HH], in_=tv[:, :HH])
            # piece list: tile 0 split into quarters so the first Exp starts
            # as soon as ~0.4 MB (not 1.57 MB) has landed; rest full tiles.
            # each piece: (tile_idx, anchor_offset_within_shard, width_T)
            pieces = [(0, q * (T // 4), T // 4) for q in range(4)]
            pieces += [(i, i * T, T) for i in range(1, NT - 1)]
            pieces += [(NT - 1, (NT - 1) * T + q * (T // 2), T // 2)
                       for q in range(2)]
            xvf = pred.rearrange("(p t) m -> p (t m)", p=P)  # whole shard view
            xbs = []
            for pidx, (ti, off, tw) in enumerate(pieces):
                xb = io.tile([P, N_CLASSES * tw], mybir.dt.bfloat16)
                if pidx == 0:
                    nc.vector.tensor_copy(out=xb, in_=xq0_f32)
                else:
                    nc.gpsimd.dma_start(
                        out=xb, in_=xvf[:, N_CLASSES * off : N_CLASSES * (off + tw)])
                xbs.append(xb)

            # one-hot planes, built per half in DVE's idle window; the odd
            # plane via mod (t2 = 1 - (c mod 2): even class ids are class 2)
            tp = stat.tile([P, N_CLASSES * TT], mybir.dt.bfloat16)
            tp3 = tp.rearrange("p (m t) -> p m t", m=N_CLASSES)
            n01 = stat.tile([P, HH], mybir.dt.bfloat16)

            def build_planes(h):
                hs = slice(h * HH, (h + 1) * HH)
                nc.vector.tensor_scalar(
                    out=tp3[:, 0, hs], in0=tg[:, hs], scalar1=1.0, scalar2=0.0,
                    op0=AluOpType.is_equal, op1=AluOpType.add)
                nc.vector.tensor_scalar(
                    out=tp3[:, 1, hs], in0=tg[:, hs], scalar1=3.0, scalar2=0.0,
                    op0=AluOpType.is_equal, op1=AluOpType.add)
                nc.vector.tensor_tensor(
                    out=n01, in0=tp3[:, 0, hs], in1=tp3[:, 1, hs],
                    op=AluOpType.add)
                nc.vector.tensor_scalar(
                    out=tp3[:, 2, hs], in0=n01, scalar1=-1.0, scalar2=1.0,
                    op0=AluOpType.mult, op1=AluOpType.add)

            build_planes(0)

            for pi, (ti, off, tw) in enumerate(pieces):
                xb = xbs[pi]
                fw = N_CLASSES * tw

                # E = exp(x) ; S = ln(E + 1) = softplus(x)  (raw logits)
                E = epool.tile([P, fw], mybir.dt.bfloat16)
                nc.scalar.activation(
                    out=E, in_=xb, func=mybir.ActivationFunctionType.Exp)
                S = spool.tile([P, fw], mybir.dt.bfloat16)
                nc.scalar.activation(
                    out=S, in_=E, func=mybir.ActivationFunctionType.Ln, bias=1.0)
                if pi == 0:
                    # second target half rides the ACT HWDGE ring so it does
                    # not steal early DMA bandwidth from tile 0's x
                    nc.scalar.dma_start(out=tg[:, HH:], in_=tv[:, HH:])
                if ti == 2 and off == 2 * T:
                    build_planes(1)

                # B = x(x-1)^2: (x-1)^2 from ACT Square on ACT_SQ tiles,
                # else a ts/tt chain on DVE (scalar_tensor_tensor is 1x-only)
                B = bpool.tile([P, fw], mybir.dt.bfloat16)
                if ti in ACT_SQ_TILES:
                    sq = scr.tile([P, fw], mybir.dt.bfloat16)
                    nc.scalar.activation(
                        out=sq, in_=xb,
                        func=mybir.ActivationFunctionType.Square, bias=-1.0)
                    nc.vector.tensor_tensor(
                        out=B, in0=xb, in1=sq, op=AluOpType.mult)
                else:
                    xm1 = scr.tile([P, fw], mybir.dt.bfloat16)
                    nc.vector.tensor_scalar(
                        out=xm1, in0=xb, scalar1=1.0, scalar2=0.0,
                        op0=AluOpType.subtract, op1=AluOpType.add)
                    b1 = scr.tile([P, fw], mybir.dt.bfloat16)
                    nc.vector.tensor_tensor(
                        out=b1, in0=xm1, in1=xb, op=AluOpType.mult)
                    nc.vector.tensor_tensor(
                        out=B, in0=xm1, in1=b1, op=AluOpType.mult)

                # XS = x*S ; M = S - B
                XS = xsp.tile([P, fw], mybir.dt.bfloat16)
                nc.vector.tensor_tensor(out=XS, in0=xb, in1=S, op=AluOpType.mult)
                M = mpool.tile([P, fw], mybir.dt.bfloat16)
                nc.vector.tensor_tensor(out=M, in0=S, in1=B, op=AluOpType.subtract)

                # strided per-class views [P, tw, m] and this piece's planes
                XS3 = XS.rearrange("p (t m) -> p t m", m=N_CLASSES)
                M3 = M.rearrange("p (t m) -> p t m", m=N_CLASSES)

                first, last = (pi == 0), (pi == len(pieces) - 1)
                # bank0: x^T @ XS (interleaved, contiguous chunks)
                for c in range(fw // MM):
                    s = slice(c * MM, (c + 1) * MM)
                    nc.tensor.matmul(
                        pb0[:, :MM], xb[:, s], XS[:, s],
                        start=(first and c == 0),
                        stop=(last and c == fw // MM - 1),
                        skip_group_check=True)
                # bank1: t^T @ XS ; bank2: t^T @ M
                for m in range(N_CLASSES):
                    for c in range(tw // MM):
                        s = slice(c * MM, (c + 1) * MM)
                        sg = slice(off + c * MM, off + (c + 1) * MM)
                        nc.tensor.matmul(
                            pb1[:, :MM], tp3[:, m, sg], XS3[:, s, m],
                            start=(first and m == 0 and c == 0),
                            stop=(last and m == N_CLASSES - 1 and c == tw // MM - 1),
                            skip_group_check=True)
                        nc.tensor.matmul(
                            pb2[:, :MM], tp3[:, m, sg], M3[:, s, m],
                            start=(first and m == 0 and c == 0),
                            stop=(last and m == N_CLASSES - 1 and c == tw // MM - 1),
                            skip_group_check=True)

            # epilogue: total = sum_k coef_k * diag(bank_k) via host mask
            dA = singles.tile([P, NB * MM], mybir.dt.float32)
            dA4 = dA.rearrange("p (k c) -> p k c", k=NB)
            cm4 = cm.rearrange("p (k c) -> p k c", k=NB)
            pbanks = [pb0, pb1, pb2]
            for k in range(NB):
                nc.vector.tensor_tensor(
                    out=dA4[:, k, :], in0=pbanks[k][:, :MM], in1=cm4[:, k, :],
                    op=AluOpType.mult)
            rA = singles.tile([P, 1], mybir.dt.float32)
            nc.vector.tensor_reduce(
                out=rA, in_=dA, axis=mybir.AxisListType.X, op=AluOpType.add)

            psT = psum.tile([1, 1], mybir.dt.float32)
            nc.tensor.matmul(psT[:, :], ones_f[:, :], rA[:, :], start=True, stop=True)
            res = singles.tile([1, 1], mybir.dt.float32)
            nc.vector.tensor_copy(out=res, in_=psT)
            nc.sync.dma_start(out=out[:], in_=res[0, :])

    nc.compile()
    return nc


_cache: dict[str, bass.Bass] = {}
last_results = None  # BassKernelResults of the most recent run (for test.py)


def _get_nc() -> bass.Bass:
    if "nc" not in _cache:
        _cache["nc"] = _build_nc()
    return _cache["nc"]


def _cmask_bf16() -> np.ndarray:
    import ml_dtypes

    coef = np.array([1.0, -2.0, 1.0], dtype=np.float32)
    m = np.zeros((P, NB * MM), dtype=np.float32)
    for k in range(NB):
        m[:, k * MM : (k + 1) * MM] = coef[k] * np.eye(P, MM, dtype=np.float32)
    return m.astype(ml_dtypes.bfloat16)


def kernel(pred: np.ndarray, targ: np.ndarray, *, trace: bool = False) -> np.ndarray:
    global last_results
    import ml_dtypes

    pred = np.ascontiguousarray(np.asarray(pred, dtype=np.float32))
    targ = np.asarray(targ)
    assert pred.shape == (N_ANCHORS, N_CLASSES), pred.shape
    assert targ.shape == (N_ANCHORS,), targ.shape

    # targ holds class ids 0..4 -- bf16 is a lossless narrowing that cuts
    # the on-device HBM read from 8.4 MB to 2.1 MB per core and keeps the
    # on-chip compares in the DVE's fast all-bf16 mode
    targb = np.ascontiguousarray(targ.astype(ml_dtypes.bfloat16))

    nc = _get_nc()
    cmask = _cmask_bf16()

    in_maps = []
    for c in range(N_CORES):
        in_maps.append({
            "pred": pred[c * N_SHARD : (c + 1) * N_SHARD],
            "targb": targb[c * N_SHARD : (c + 1) * N_SHARD],
            "cmask": cmask,
        })

    res = bass_utils.run_bass_kernel_spmd(
        nc, in_maps, core_ids=list(range(N_CORES)), trace=trace
    )
    last_results = res

    total = np.float64(0.0)
    for r in res.results:
        total += np.float64(r["out"][0])
    mean = total / (N_ANCHORS * N_CLASSES)
    return np.float32(mean)


# revision 41
# speedup vs baseline: 1.1980x; 1.0061x over previous
"""Trainium2 Bass kernel for nn_BCE_Loss (focal-style BCE-with-logits, mean).

Reference math per anchor row x[0:3] (logits) and integer target c:
    col = 0 if c==1 else 1 if c==3 else 2
    t   = one_hot(col, 3)
    w   = (1-pt)^2,  pt = x*t + (1-x)*(1-t)        [from detached logits]
    bce = max(x,0) - x*t + log1p(exp(-|x|))
    out = mean(w * bce)

Per element the reference equals u^2 * softplus(v) with v = x*(1-2t),
u = v + t.  Two exact identities (for t in {0,1}) remove every
target-dependent elementwise tensor:
    u^2 = x^2 + u - x          (u(u-1) == x(x-1) in both branches)
    softplus(v) = S - x*t,     S := softplus(x)   (sp(x)-sp(-x) == x)
which collapse the loss to
    sum loss = sum x^2*S  +  sum_am t[a,m] * q(x[a,m])
    q(x) = (1-2x)*S - x*(x-1)^2 = -2*(x*S) + (S - x*(x-1)^2)
so the activation path runs on RAW logits (no dependency on the target
path) and the one-hot enters only through per-class planes t_m [P,T]
built contiguously (never a strided interleaved write).

Layout/engine split per core (NT tiles of [P=128, F=3T]):
    one-hot planes for the WHOLE shard are built up front from one bf16
    target load: t0=(c==1), t1=(c==3), t2=(1-t0)-t1  (3 DVE ops total)
    per tile:
      ACT:  E = Exp(x), S = Ln(E+1)          [+ (x-1)^2 on ACT_SQ tiles]
      DVE:  XS = x*S,  B = x(x-1)^2,  M = S - B
      PE:   three accumulating diag-trick banks (one PSUM bank each --
            start=True clears a whole physical bank):
              bank0 += x_c^T @ XS_c      (diag: sum x^2 S)
              bank1 += t_mc^T @ XS_mc    (diag: sum t x S, strided moving)
              bank2 += t_mc^T @ M_mc     (diag: sum t (S - B))
            total = diag0 - 2*diag1 + diag2 via a host coefficient mask.

HBM traffic per core: pred 12.58 MB (f32, cast to bf16 in the SWDGE DMA
datapath) + targ 2.1 MB (host-narrowed int64 -> bf16; values 0..4 are
exact in bf16) ~= 14.7 MB.

Sharding: pure data-parallel across 8 NeuronCores -- each core takes a
contiguous block of anchors; per-core output is a single partial sum; the
host sums the 8 partials and divides by the element count.
"""

import numpy as np

import concourse.bacc as bacc
import concourse.bass as bass
import concourse.mybir as mybir
from concourse import bass_utils
from concourse.alu_op_type import AluOpType
from concourse.tile import TileContext

N_CORES = 8
N_ANCHORS = 8388608
N_CLASSES = 3
N_SHARD = N_ANCHORS // N_CORES  # 1048576
P = 128  # SBUF partitions
T = 1024  # anchor rows per partition per tile
NT = N_SHARD // (P * T)  # 8 tiles per core
F = N_CLASSES * T  # free dim of an x tile (3072)
TT = NT * T  # anchors per partition for the whole shard (8192)
MM = 128  # diag-trick matmul chunk width
NB = 3  # psum banks (coefficient groups)
# tiles whose (x-1)^2 runs on ACT (A/B-tuned): tile 0's quarters' small
# squares fill the early ACT starvation gaps, mid-tiles {1,3,5} balance
# DVE vs ACT, and tile 7 keeps its halves from leaving a DVE chain
# dangling in the drain
ACT_SQ_TILES = {0, 1, 3, 5, 7}


class _Bacc(bacc.Bacc):
    """Bacc with the ACT table pinned to natural_log_exp_and_others.

    The default chooser puts Exp in exp_and_others and Ln in natural_log,
    reloading tables every tile (~1.3us each). Exp, Ln and Square all live
    in natural_log_exp_and_others; emptying every other set (positions
    kept -- act_func_set_id is the index into act_info.json) forces one
    load."""

    _ACT_SET = "natural_log_exp_and_others"

    def insert_act_table_loads(self):
        import bass_rust as _bass_rust

        from concourse.hw_specs import get_activation_tables

        has_activation = any(
            isinstance(i, mybir.InstActivation)
            for b in self.main_func.blocks
            for i in b.instructions
        )
        if not has_activation:
            return
        tables = [
            (name, (fns if name == self._ACT_SET else set()))
            for name, fns in get_activation_tables(self.m.arch).items()
        ]
        _bass_rust.insert_act_table_loads(self, tables)


def _build_nc() -> bass.Bass:
    nc = _Bacc("TRN2", target_bir_lowering=False, num_swdge_queues=4)
    # the Square(x, bias=-1) activation needs a -1.0 const AP; only 0/1 ship
    _m1 = nc.alloc_sbuf_tensor("const-float32--1.0", [128, 1], mybir.dt.float32)
    nc.gpsimd.memset(_m1.ap(), -1.0)
    nc.const_aps.aps[(mybir.dt.float32, -1.0)] = _m1.ap()
    pred = nc.dram_tensor(
        "pred", [N_SHARD, N_CLASSES], mybir.dt.float32, kind="ExternalInput"
    )
    targ = nc.dram_tensor("targb", [N_SHARD], mybir.dt.bfloat16, kind="ExternalInput")
    cmask = nc.dram_tensor(
        "cmask", [P, NB * MM], mybir.dt.bfloat16, kind="ExternalInput"
    )
    out = nc.dram_tensor("out", [1], mybir.dt.float32, kind="ExternalOutput")

    xv = pred.rearrange("(n p t) m -> n p (t m)", p=P, t=T)
    tv = targ.rearrange("(p t) -> p t", p=P)  # [P, TT], whole shard

    n_mm = F // MM  # interleaved chunks per tile (24)
    n_tm = T // MM  # per-plane chunks per tile (8)

    with TileContext(nc) as tc:
        with (
            tc.tile_pool(name="io", bufs=5) as io,
            tc.tile_pool(name="stat", bufs=1) as stat,
            tc.tile_pool(name="epool", bufs=2) as epool,
            tc.tile_pool(name="spool", bufs=2) as spool,
            tc.tile_pool(name="xsp", bufs=3) as xsp,
            tc.tile_pool(name="mpool", bufs=3) as mpool,
            tc.tile_pool(name="bpool", bufs=2) as bpool,
            tc.tile_pool(name="scratch", bufs=1) as scr,
            tc.tile_pool(name="singles", bufs=1) as singles,
            tc.tile_pool(name="psum", bufs=1, space="PSUM") as psum,
        ):
            ones_f = singles.tile([P, 1], mybir.dt.float32)
            nc.vector.memset(ones_f, 1.0)
            pb0 = psum.tile([P, 512], mybir.dt.float32)
            pb1 = psum.tile([P, 512], mybir.dt.float32)
            pb2 = psum.tile([P, 512], mybir.dt.float32)

            # target load first (split in two halves so the first half's
            # one-hot planes are ready before tile 0's T-bank matmuls), then
            # every x cast-DMA trigger
            # the very first x quarter rides the HWDGE sync ring (which gets
            # its first packet out ~3us earlier than SWDGE) as raw f32 and is
            # cast to bf16 by the otherwise-idle DVE
            xq0_f32 = stat.tile([P, N_CLASSES * (T // 4)], mybir.dt.float32)
            nc.sync.dma_start(out=xq0_f32, in_=pred.rearrange(
                "(p t) m -> p (t m)", p=P)[:, : N_CLASSES * (T // 4)])
            cm = singles.tile([P, NB * MM], mybir.dt.bfloat16)
            nc.sync.dma_start(out=cm, in_=cmask[:, :])
            tg = stat.tile([P, TT], mybir.dt.bfloat16)
            HH = TT // 2
            nc.sync.dma_start(out=tg[:, :HH], in_=tv[:, :HH])
            # piece list: tile 0 split into quarters so the first Exp starts
            # as soon as ~0.4 MB (not 1.57 MB) has landed; rest full tiles.
            # each piece: (tile_idx, anchor_offset_within_shard, width_T)
            pieces = [(0, q * (T // 4), T // 4) for q in range(4)]
            pieces += [(i, i * T, T) for i in range(1, NT - 1)]
            pieces += [(NT - 1, (NT - 1) * T + q * (T // 2), T // 2)
                       for q in range(2)]
            xvf = pred.rearrange("(p t) m -> p (t m)", p=P)  # whole shard view
            xbs = []
            for pidx, (ti, off, tw) in enumerate(pieces):
                xb = io.tile([P, N_CLASSES * tw], mybir.dt.bfloat16)
                if pidx == 0:
                    nc.vector.tensor_copy(out=xb, in_=xq0_f32)
                else:
                    nc.gpsimd.dma_start(
                        out=xb, in_=xvf[:, N_CLASSES * off : N_CLASSES * (off + tw)])
                xbs.append(xb)

            # one-hot planes, built per half in DVE's idle window; the odd
            # plane via mod (t2 = 1 - (c mod 2): even class ids are class 2)
            tp = stat.tile([P, N_CLASSES * TT], mybir.dt.bfloat16)
            tp3 = tp.rearrange("p (m t) -> p m t", m=N_CLASSES)
            n01 = stat.tile([P, HH], mybir.dt.bfloat16)

            def build_planes(h):
                hs = slice(h * HH, (h + 1) * HH)
                nc.vector.tensor_scalar(
                    out=tp3[:, 0, hs], in0=tg[:, hs], scalar1=1.0, scalar2=0.0,
                    op0=AluOpType.is_equal, op1=AluOpType.add)
                nc.vector.tensor_scalar(
                    out=tp3[:, 1, hs], in0=tg[:, hs], scalar1=3.0, scalar2=0.0,
                    op0=AluOpType.is_equal, op1=AluOpType.add)
                nc.vector.tensor_tensor(
                    out=n01, in0=tp3[:, 0, hs], in1=tp3[:, 1, hs],
                    op=AluOpType.add)
                nc.vector.tensor_scalar(
                    out=tp3[:, 2, hs], in0=n01, scalar1=-1.0, scalar2=1.0,
                    op0=AluOpType.mult, op1=AluOpType.add)

            build_planes(0)

            for pi, (ti, off, tw) in enumerate(pieces):
                xb = xbs[pi]
                fw = N_CLASSES * tw

                # E = exp(x) ; S = ln(E + 1) = softplus(x)  (raw logits)
                E = epool.tile([P, fw], mybir.dt.bfloat16)
                nc.scalar.activation(
                    out=E, in_=xb, func=mybir.ActivationFunctionType.Exp)
                S = spool.tile([P, fw], mybir.dt.bfloat16)
                nc.scalar.activation(
                    out=S, in_=E, func=mybir.ActivationFunctionType.Ln, bias=1.0)
                if pi == 0:
                    # second target half rides the ACT HWDGE ring so it does
                    # not steal early DMA bandwidth from tile 0's x
                    nc.scalar.dma_start(out=tg[:, HH:], in_=tv[:, HH:])
                if ti == 2 and off == 2 * T:
                    build_planes(1)

                # B = x(x-1)^2: (x-1)^2 from ACT Square on ACT_SQ tiles,
                # else a ts/tt chain on DVE (scalar_tensor_tensor is 1x-only)
                B = bpool.tile([P, fw], mybir.dt.bfloat16)
                if ti in ACT_SQ_TILES:
                    sq = scr.tile([P, fw], mybir.dt.bfloat16)
                    nc.scalar.activation(
                        out=sq, in_=xb,
                        func=mybir.ActivationFunctionType.Square, bias=-1.0)
                    nc.vector.tensor_tensor(
                        out=B, in0=xb, in1=sq, op=AluOpType.mult)
                else:
                    xm1 = scr.tile([P, fw], mybir.dt.bfloat16)
                    nc.vector.tensor_scalar(
                        out=xm1, in0=xb, scalar1=1.0, scalar2=0.0,
                        op0=AluOpType.subtract, op1=AluOpType.add)
                    b1 = scr.tile([P, fw], mybir.dt.bfloat16)
                    nc.vector.tensor_tensor(
                        out=b1, in0=xm1, in1=xb, op=AluOpType.mult)
                    nc.vector.tensor_tensor(
                        out=B, in0=xm1, in1=b1, op=AluOpType.mult)

                # XS = x*S ; M = S - B
                XS = xsp.tile([P, fw], mybir.dt.bfloat16)
                nc.vector.tensor_tensor(out=XS, in0=xb, in1=S, op=AluOpType.mult)
                M = mpool.tile([P, fw], mybir.dt.bfloat16)
                nc.vector.tensor_tensor(out=M, in0=S, in1=B, op=AluOpType.subtract)

                # strided per-class views [P, tw, m] and this piece's planes
                XS3 = XS.rearrange("p (t m) -> p t m", m=N_CLASSES)
                M3 = M.rearrange("p (t m) -> p t m", m=N_CLASSES)

                first, last = (pi == 0), (pi == len(pieces) - 1)
                # bank0: x^T @ XS (interleaved, contiguous chunks)
                for c in range(fw // MM):
                    s = slice(c * MM, (c + 1) * MM)
                    nc.tensor.matmul(
                        pb0[:, :MM], xb[:, s], XS[:, s],
                        start=(first and c == 0),
                        stop=(last and c == fw // MM - 1),
                        skip_group_check=True)
                # bank1: t^T @ XS ; bank2: t^T @ M
                for m in range(N_CLASSES):
                    for c in range(tw // MM):
                        s = slice(c * MM, (c + 1) * MM)
                        sg = slice(off + c * MM, off + (c + 1) * MM)
                        nc.tensor.matmul(
                            pb1[:, :MM], tp3[:, m, sg], XS3[:, s, m],
                            start=(first and m == 0 and c == 0),
                            stop=(last and m == N_CLASSES - 1 and c == tw // MM - 1),
                            skip_group_check=True)
                        nc.tensor.matmul(
                            pb2[:, :MM], tp3[:, m, sg], M3[:, s, m],
                            start=(first and m == 0 and c == 0),
                            stop=(last and m == N_CLASSES - 1 and c == tw // MM - 1),
                            skip_group_check=True)

            # epilogue: total = sum_k coef_k * diag(bank_k) via host mask
            dA = singles.tile([P, NB * MM], mybir.dt.float32)
            dA4 = dA.rearrange("p (k c) -> p k c", k=NB)
            cm4 = cm.rearrange("p (k c) -> p k c", k=NB)
            pbanks = [pb0, pb1, pb2]
            for k in range(NB):
                nc.vector.tensor_tensor(
                    out=dA4[:, k, :], in0=pbanks[k][:, :MM], in1=cm4[:, k, :],
                    op=AluOpType.mult)
            rA = singles.tile([P, 1], mybir.dt.float32)
            nc.vector.tensor_reduce(
                out=rA, in_=dA, axis=mybir.AxisListType.X, op=AluOpType.add)

            psT = psum.tile([1, 1], mybir.dt.float32)
            nc.tensor.matmul(psT[:, :], ones_f[:, :], rA[:, :], start=True, stop=True)
            res = singles.tile([1, 1], mybir.dt.float32)
            nc.vector.tensor_copy(out=res, in_=psT)
            nc.sync.dma_start(out=out[:], in_=res[0, :])

    nc.compile()
    return nc


_cache: dict[str, bass.Bass] = {}
last_results = None  # BassKernelResults of the most recent run (for test.py)


def _get_nc() -> bass.Bass:
    if "nc" not in _cache:
        _cache["nc"] = _build_nc()
    return _cache["nc"]


def _cmask_bf16() -> np.ndarray:
    import ml_dtypes

    coef = np.array([1.0, -2.0, 1.0], dtype=np.float32)
    m = np.zeros((P, NB * MM), dtype=np.float32)
    for k in range(NB):
        m[:, k * MM : (k + 1) * MM] = coef[k] * np.eye(P, MM, dtype=np.float32)
    return m.astype(ml_dtypes.bfloat16)


def kernel(pred: np.ndarray, targ: np.ndarray, *, trace: bool = False) -> np.ndarray:
    global last_results
    import ml_dtypes

    pred = np.ascontiguousarray(np.asarray(pred, dtype=np.float32))
    targ = np.asarray(targ)
    assert pred.shape == (N_ANCHORS, N_CLASSES), pred.shape
    assert targ.shape == (N_ANCHORS,), targ.shape

    # targ holds class ids 0..4 -- bf16 is a lossless narrowing that cuts
    # the on-device HBM read from 8.4 MB to 2.1 MB per core and keeps the
    # on-chip compares in the DVE's fast all-bf16 mode
    targb = np.ascontiguousarray(targ.astype(ml_dtypes.bfloat16))

    nc = _get_nc()
    cmask = _cmask_bf16()

    in_maps = []
    for c in range(N_CORES):
        in_maps.append({
            "pred": pred[c * N_SHARD : (c + 1) * N_SHARD],
            "targb": targb[c * N_SHARD : (c + 1) * N_SHARD],
            "cmask": cmask,
        })

    res = bass_utils.run_bass_kernel_spmd(
        nc, in_maps, core_ids=list(range(N_CORES)), trace=trace
    )
    last_results = res

    total = np.float64(0.0)
    for r in res.results:
        total += np.float64(r["out"][0])
    mean = total / (N_ANCHORS * N_CLASSES)
    return np.float32(mean)
